# revision 15
# baseline (speedup 1.0000x reference)
"""Segment-softmax GNN attention kernel for 8 Trainium2 NeuronCores.

Math (reference): latent = leaky_relu(x @ W + b, 0.2)  -> [E, 1]
                  out = scatter_softmax(latent, index) -> [E, 1]

Strategy v7 (PE matvec + partition-local scans; no indirect DMA):
  Host: stable-sort edges by destination segment; shard segment-aligned
  across 8 cores (6250 segments each => no cross-core reduction); pad
  every segment to a multiple of G=4 edges (dummy x columns whose logit
  is ~-500 -> exp == 0).  Segments are then BIN-PACKED sequentially
  into the 128 partitions (cap J slots each), so no segment crosses a
  partition boundary and the segment softmax needs no cross-partition
  halo at all.  Edges are stored TRANSPOSED and in bf16: xsT[f, col]
  with col = t*EDGE_TILE + c*128 + p for padded position P = p*J +
  t*CPT + c.  Each [128,128] chunk of a tile is the stationary operand
  of a PE matmul (moving = W[128,1]); z[p, c] lands in partition-major
  padded order and 4-slot block sums are partition-contiguous.
  Device per core, all static APs:
    A) stream xT tiles on a greedy schedule over the three DMA queues
       (sync/scalar HW-DGE ~165 GB/s, gpsimd SW ~100); 32 PE
       matmuls/tile -> z in PSUM; DVE leaky = max(z+b, 0.2z+0.2b);
       scalar Exp -> e in SBUF; DVE 4-slot block sums -> b4_sb.
    B) per-partition segment denominators straight from SBUF: forward
       within-segment prefix scan (state = notstart*state + b4) and a
       reversed max-carry scan (state = (notend*state) max fwd)
       broadcast each segment total; reciprocal_approx_fast -> r4.
       Done in two column halves; the left half (+-11-block overlap at
       the split) runs while phase A still streams.
    C) out = e * r4[block]; two half DMAs (left overlaps phase A).
  Host: drop padding, inverse-permute.
  No max-subtraction needed: logits ~ N(0,1) so exp is safe in f32.
"""

import os
import sys

sys.path.insert(0, "/opt/trn_rl_repo")

import numpy as np
import ml_dtypes

BF16 = ml_dtypes.bfloat16

N_NODES = 50000
N_CORES = 8
SEG_PER_CORE = N_NODES // N_CORES          # 6250
D = 128
EDGE_TILE = 4096                           # edges per phase-A tile
CPT = EDGE_TILE // 128                     # 32 slots per partition per tile
XCH = 4                                    # tile-0 fast-start split
G = 4                                      # block granularity (segment pad)
NEG_SLOPE = 0.2
HB = 11                                    # split overlap (segment <= 12 blocks)
DUMMY_Z = -500.0                           # dummy-edge logit target

_compiled_cache = {}


def _build_graph(E_pad: int):
    import concourse.bacc as bacc
    import concourse.tile as tile
    from concourse import bass, mybir

    f32 = mybir.dt.float32
    bf16 = mybir.dt.bfloat16
    n_xt = E_pad // EDGE_TILE
    J = E_pad // 128                       # slots per partition
    BCOL = J // G                          # blocks per partition
    CB = CPT // G                          # blocks per partition per tile (8)

    # column split for phase-B/C overlap: left [0, SPL), right [SPL, BCOL)
    SPL = ((BCOL // 2) // CB) * CB         # multiple of CB (tile block width)
    LW = SPL + HB + 1                      # left scan window [0, LW)
    RW0 = SPL - HB                         # right scan window [RW0, BCOL)
    LTILE = (LW + CB - 1) // CB            # tiles needed for left window
    RWN = BCOL - RW0

    nc = bacc.Bacc("TRN2", target_bir_lowering=False, debug=False,
                   num_devices=N_CORES)

    xs_d = nc.dram_tensor("xst", [128, E_pad], bf16, kind="ExternalInput")
    w_d = nc.dram_tensor("wcol", [128, 1], bf16, kind="ExternalInput")
    b_d = nc.dram_tensor("bvec", [1, 1], f32, kind="ExternalInput")
    b02_d = nc.dram_tensor("b02", [1, 1], f32, kind="ExternalInput")
    ns_d = nc.dram_tensor("nstart", [128, BCOL], bf16, kind="ExternalInput")
    ne_d = nc.dram_tensor("nend", [128, BCOL], bf16, kind="ExternalInput")
    out_d = nc.dram_tensor("out", [E_pad, 1], f32, kind="ExternalOutput")

    AP = bass.AP
    ALU = mybir.AluOpType
    ACT = mybir.ActivationFunctionType

    def rev(ap):
        """Reversed-free-dim view of a [128, F] AP."""
        (sp, np_), (sf, nf) = ap.ap
        return AP(tensor=ap.tensor, offset=ap.offset + sf * (nf - 1),
                  ap=[[sp, np_], [-sf, nf]])

    # greedy DMA schedule for tiles 1..n_xt-1 over the three queues
    rates = {"sync": 0.165, "scalar": 0.165, "gps": 0.100}  # MB/us
    load = {"sync": 0.25, "scalar": 0.5, "gps": 0.3}        # minis + consts
    sched = []
    for _ in range(1, n_xt):
        q = min(rates, key=lambda q: (load[q] + 1.0) / rates[q])
        load[q] += 1.0
        sched.append(q)

    with tile.TileContext(nc) as tc:
        with (
            tc.tile_pool(name="consts", bufs=1) as consts,
            tc.tile_pool(name="xin", bufs=8) as xin,
            tc.tile_pool(name="small", bufs=4) as small,
            tc.tile_pool(name="keep", bufs=1) as keep,
            tc.tile_pool(name="bwork", bufs=1) as bwork,
            tc.tile_pool(name="zp", bufs=2, space="PSUM") as zp,
        ):
            # --- constants (wb first: needed by the first matmul) ---
            wb = consts.tile([128, 1], bf16)
            nc.gpsimd.dma_start(out=wb[:], in_=w_d[:, :])
            bb = consts.tile([128, 1], f32)
            nc.gpsimd.dma_start(
                out=bb[:], in_=AP(tensor=b_d, offset=0, ap=[[0, 128], [1, 1]])
            )
            bb02 = consts.tile([128, 1], f32)
            nc.gpsimd.dma_start(
                out=bb02[:], in_=AP(tensor=b02_d, offset=0, ap=[[0, 128], [1, 1]])
            )

            e4_sb = keep.tile([128, J], f32)       # all exp values, SBUF-resident
            b4_sb = keep.tile([128, BCOL], f32)    # all block sums, SBUF-resident
            out_sb = keep.tile([128, J], f32)

            # tile 0 split into 4 independent mini-tiles for a fast start
            csz = EDGE_TILE // XCH
            t0q = [nc.sync, nc.scalar, nc.gpsimd, nc.scalar]
            xt0 = []
            for ch in range(XCH):
                mt = xin.tile([128, csz], bf16, tag=f"mini{ch}")
                t0q[ch].dma_start(
                    out=mt[:],
                    in_=AP(tensor=xs_d, offset=ch * csz,
                           ap=[[E_pad, 128], [1, csz]]),
                )
                xt0.append(mt)
            nsm = consts.tile([128, BCOL], bf16)
            nc.sync.dma_start(out=nsm[:], in_=ns_d[:, :])
            nem = consts.tile([128, BCOL], bf16)
            nc.sync.dma_start(out=nem[:], in_=ne_d[:, :])

            qmap = {"sync": nc.sync, "scalar": nc.scalar, "gps": nc.gpsimd}

            def process(xt, slot0, nsl):
                """matvec+leaky+exp+blocksum for nsl slots starting at slot0."""
                zt = zp.tile([128, nsl], f32, tag=f"z{nsl}")
                for c in range(nsl):
                    nc.tensor.matmul(
                        zt[:, c:c + 1],
                        xt[:, c * 128:(c + 1) * 128],
                        wb[:],
                    )
                ut = small.tile([128, nsl], f32, tag=f"u{nsl}")
                nc.vector.tensor_scalar(out=ut[:], in0=zt[:], scalar1=NEG_SLOPE,
                                        scalar2=bb02[:, 0:1], op0=ALU.mult,
                                        op1=ALU.add)
                lt = small.tile([128, nsl], f32, tag=f"l{nsl}")
                nc.vector.scalar_tensor_tensor(out=lt[:], in0=zt[:],
                                               scalar=bb[:, 0:1], in1=ut[:],
                                               op0=ALU.add, op1=ALU.max)
                et = e4_sb[:, slot0:slot0 + nsl]
                nc.scalar.activation(out=et, in_=lt[:], func=ACT.Exp)
                nc.vector.tensor_reduce(
                    out=b4_sb[:, slot0 // G:(slot0 + nsl) // G],
                    in_=et.rearrange("p (cb g) -> p cb g", g=G),
                    axis=mybir.AxisListType.X, op=ALU.add)

            def seg_denom(w0, w1, d0, d1, tag):
                """Scans over window [w0,w1); denominators for blocks [d0,d1);
                out = e*r4 for those blocks + output DMA."""
                wn = w1 - w0
                fwd = bwork.tile([128, wn], f32, tag=f"f{tag}")
                nc.vector.tensor_tensor_scan(
                    out=fwd[:], data0=nsm[:, w0:w1], data1=b4_sb[:, w0:w1],
                    initial=0.0, op0=ALU.mult, op1=ALU.add)
                d4 = bwork.tile([128, wn], f32, tag=f"d{tag}")
                nc.vector.tensor_tensor_scan(
                    out=rev(d4[:]), data0=rev(nem[:, w0:w1]),
                    data1=rev(fwd[:]), initial=0.0,
                    op0=ALU.mult, op1=ALU.max)
                dn = d1 - d0
                d4e = bwork.tile([128, dn], f32, tag=f"e{tag}")
                nc.vector.tensor_scalar(out=d4e[:], in0=d4[:, d0 - w0:d1 - w0],
                                        scalar1=1e-12, scalar2=None,
                                        op0=ALU.add)
                r4 = bwork.tile([128, dn], f32, tag=f"r{tag}")
                nc.vector.reciprocal_approx_fast(out=r4[:], in_=d4e[:])
                r4_ap = r4[:]
                r4b = AP(tensor=r4_ap.tensor, offset=r4_ap.offset,
                         ap=[r4_ap.ap[0], r4_ap.ap[1], [0, G]])
                s0, s1 = d0 * G, d1 * G
                oh = out_sb[:, s0:s1]
                nc.vector.tensor_tensor(
                    out=oh.rearrange("p (cb g) -> p cb g", g=G),
                    in0=e4_sb[:, s0:s1].rearrange("p (cb g) -> p cb g", g=G),
                    in1=r4b, op=ALU.mult)
                qmap[tag].dma_start(
                    out=AP(tensor=out_d, offset=s0, ap=[[J, 128], [1, s1 - s0]]),
                    in_=oh,
                )

            # --- phase A + overlapped left-half phase B/C ---
            for ch in range(XCH):
                process(xt0[ch], ch * (csz // 128), csz // 128)
            for i in range(1, n_xt):
                xt = xin.tile([128, EDGE_TILE], bf16)
                qmap[sched[i - 1]].dma_start(
                    out=xt[:],
                    in_=AP(tensor=xs_d, offset=i * EDGE_TILE,
                           ap=[[E_pad, 128], [1, EDGE_TILE]]),
                )
                process(xt[:], i * CPT, CPT)
                if i == LTILE:
                    seg_denom(0, LW, 0, SPL, "scalar")
            seg_denom(RW0, BCOL, SPL, BCOL, "sync")

    nc.compile()
    return nc


def _host_prep(x, W, b, index):
    """Sort/pad/bin-pack/shard on host; per-core in_maps + reassembly info."""
    x = np.ascontiguousarray(np.asarray(x, dtype=np.float32))
    W = np.asarray(W, dtype=np.float32).reshape(D)
    b = np.asarray(b, dtype=np.float32).reshape(1)
    idx = np.asarray(index).astype(np.int64).ravel()
    E = idx.shape[0]

    order = np.argsort(idx, kind="stable")
    idx_s = idx[order]
    counts = np.bincount(idx_s, minlength=N_NODES).astype(np.int64)
    seg_starts = np.zeros(N_NODES + 1, dtype=np.int64)
    np.cumsum(counts, out=seg_starts[1:])
    plen = ((counts + G - 1) // G) * G                     # padded lengths

    core_e = seg_starts[np.arange(N_CORES + 1) * SEG_PER_CORE]

    # per-core sequential bin-packing of segments into 128 partitions
    # (capacity J); J grown in EDGE_TILE/128 steps until 128 bins suffice
    J = 800
    packs = None
    while True:
        packs = []
        ok = True
        for k in range(N_CORES):
            s0 = k * SEG_PER_CORE
            pl = plen[s0:s0 + SEG_PER_CORE]
            binid = np.empty(SEG_PER_CORE, dtype=np.int64)
            off = np.empty(SEG_PER_CORE, dtype=np.int64)
            bi, fill = 0, 0
            for s in range(SEG_PER_CORE):
                L = int(pl[s])
                if fill + L > J:
                    bi += 1
                    fill = 0
                binid[s] = bi
                off[s] = fill
                fill += L
            if bi >= 128:
                ok = False
                break
            packs.append((binid, off))
        if ok:
            break
        J += EDGE_TILE // 128

    E_pad = 128 * J
    x_sorted = x[order]
    wcol = W.reshape(128, 1).astype(BF16)
    bvec = b.reshape(1, 1).astype(np.float32)
    b02 = (NEG_SLOPE * b).reshape(1, 1).astype(np.float32)
    wsq = float(W @ W)
    dummy_col = ((DUMMY_Z / max(wsq, 1e-30)) * W).astype(BF16)  # logit ~ -500

    BCOL = J // G

    # padded position P = p*J + t*CPT + c  ->  xsT column t*EDGE_TILE + c*128 + p
    Pv = np.arange(E_pad, dtype=np.int64)
    colmap = ((Pv % J) // CPT) * EDGE_TILE + (Pv % CPT) * 128 + (Pv // J)

    in_maps = []
    reasm = []
    for k in range(N_CORES):
        e0, e1 = int(core_e[k]), int(core_e[k + 1])
        cnt = e1 - e0
        s0 = k * SEG_PER_CORE
        binid, off = packs[k]
        sstart = seg_starts[s0:s0 + SEG_PER_CORE] - e0     # compact local starts

        seg_local = (idx_s[e0:e1] - s0).astype(np.int64)
        pos_in_seg = np.arange(cnt, dtype=np.int64) - sstart[seg_local]
        ppos = binid[seg_local] * J + off[seg_local] + pos_in_seg

        xst = np.empty((128, E_pad), dtype=BF16)
        xst[:] = dummy_col[:, None]
        xst[:, colmap[ppos]] = x_sorted[e0:e1].astype(BF16).T

        # per-partition block -> segment id (unique ids for dummy blocks)
        bseg = np.full(128 * BCOL, -1, dtype=np.int64)
        pl = plen[s0:s0 + SEG_PER_CORE]
        blk0 = (binid * J + off) // G                      # first block of seg
        nblk = pl // G
        bidx = np.repeat(blk0 + np.cumsum(nblk) - nblk, 0)  # placeholder
        # fill segment ids block-wise
        rep_seg = np.repeat(np.arange(SEG_PER_CORE), nblk)
        rep_blk = np.repeat(blk0, nblk) + (
            np.arange(int(nblk.sum()), dtype=np.int64)
            - np.repeat(np.cumsum(nblk) - nblk, nblk))
        bseg[rep_blk] = rep_seg
        dummy_mask = bseg < 0
        bseg[dummy_mask] = SEG_PER_CORE + np.arange(int(dummy_mask.sum()))
        bseg2 = bseg.reshape(128, BCOL)
        notstart = np.ones((128, BCOL), np.float32)
        notstart[:, 1:] = (bseg2[:, 1:] == bseg2[:, :-1])
        notstart[:, 0] = 0.0
        notend = np.ones((128, BCOL), np.float32)
        notend[:, :-1] = (bseg2[:, :-1] == bseg2[:, 1:])
        notend[:, -1] = 0.0

        in_maps.append({
            "xst": xst, "wcol": wcol, "bvec": bvec, "b02": b02,
            "nstart": notstart.astype(BF16), "nend": notend.astype(BF16),
        })
        reasm.append(ppos)

    return in_maps, reasm, order, core_e, E_pad, E


def _emulate_core(m, E_pad):
    """Numpy emulation of the device graph for one core (host-logic check)."""
    xst, wcol, bvec = m["xst"], m["wcol"], m["bvec"]
    nsm = m["nstart"].astype(np.float32)
    nem = m["nend"].astype(np.float32)
    J = E_pad // 128
    BCOL = J // G
    z_cols = (xst.astype(np.float32).T @ wcol.astype(np.float32)).ravel()
    cv = np.arange(E_pad, dtype=np.int64)
    t, rc = cv // EDGE_TILE, cv % EDGE_TILE
    c, p = rc // 128, rc % 128
    P = p * J + t * CPT + c
    z = np.empty(E_pad, dtype=np.float32)
    z[P] = z_cols
    b0 = bvec.ravel()[0]
    zb = z + b0
    l = np.where(zb >= 0, zb, NEG_SLOPE * zb)
    e = np.exp(l).astype(np.float32).reshape(128, J)
    B4 = e.reshape(128, BCOL, G).sum(axis=2)
    # emulate the split-window scans exactly as the device does
    CB = CPT // G
    SPL = ((BCOL // 2) // CB) * CB
    LW = SPL + HB + 1
    RW0 = SPL - HB
    D4 = np.zeros((128, BCOL), np.float32)
    for (w0, w1, d0, d1) in [(0, LW, 0, SPL), (RW0, BCOL, SPL, BCOL)]:
        fwd = np.zeros((128, w1 - w0), np.float32)
        st = np.zeros(128, np.float32)
        for tt in range(w1 - w0):
            st = nsm[:, w0 + tt] * st + B4[:, w0 + tt]
            fwd[:, tt] = st
        d4 = np.zeros((128, w1 - w0), np.float32)
        st = np.zeros(128, np.float32)
        for tt in range(w1 - w0 - 1, -1, -1):
            st = np.maximum(nem[:, w0 + tt] * st, fwd[:, tt])
            d4[:, tt] = st
        D4[:, d0:d1] = d4[:, d0 - w0:d1 - w0]
    R4 = 1.0 / (D4 + 1e-12)
    out = e * np.repeat(R4, G, axis=1)
    return out.reshape(-1).astype(np.float32)


LAST_RESULTS = None  # BassKernelResults from the most recent run


def kernel(x, W, b, index):
    global LAST_RESULTS
    in_maps, reasm, order, core_e, E_pad, E = _host_prep(x, W, b, index)

    if os.environ.get("KERNEL_EMULATE"):
        outs = [_emulate_core(m, E_pad) for m in in_maps]
    else:
        from concourse.bass_utils import run_bass_kernel_spmd

        if E_pad not in _compiled_cache:
            _compiled_cache[E_pad] = _build_graph(E_pad)
        nc = _compiled_cache[E_pad]
        trace = bool(os.environ.get("BASS_TRACE"))
        LAST_RESULTS = run_bass_kernel_spmd(
            nc, in_maps, list(range(N_CORES)), trace=trace,
        )
        outs = [r["out"] for r in LAST_RESULTS.results]

    out_sorted = np.empty(E, dtype=np.float32)
    for k in range(N_CORES):
        e0, e1 = int(core_e[k]), int(core_e[k + 1])
        out_sorted[e0:e1] = np.asarray(outs[k]).ravel()[reasm[k]]
    out = np.empty(E, dtype=np.float32)
    out[order] = out_sorted
    return out[:, None]
